# revision 15
# baseline (speedup 1.0000x reference)
"""Trainium2 Bass kernel for nn_GRUDecoder: 8-core data-parallel GRU decoder.

Structure (per core, batch shard of 256 rows):
  - All recurrence tensors live as [gate/H partitions, batch free] so the
    per-step matmul rhs is the hidden state directly (no transposes in loop).
  - h0 = tanh(W_lh @ latent^T + b_lh); gi = W_ih @ h0 + (b_ih [+ b_hh for r,z])
    precomputed once on device.
  - 512 sequential GRU steps; gi_rz is folded into the gate matmul via an
    identity-matmul PSUM accumulation; b_hh_n is folded into the r*ghn multiply
    via a fused scalar_tensor_tensor op.
  - Head projections (recon/sport/dport/proto) are computed per step as
    out[batch, head_dim] = h_chunk.T @ W_head^T, accumulated in PSUM over
    small step groups (head biases added via a K=1 ones-matmul), copied to
    SBUF staging, and DMA'd to HBM in batched transfers with >=512B runs.
  - All constants ride in one packed [128, NCOL] tensor -> single input DMA
    (one DMA semaphore; matmul instructions only support one sync wait).
"""

import os
import sys

import numpy as np

if "/opt/trn_rl_repo/concourse" not in sys.path:
    sys.path.insert(0, "/opt/trn_rl_repo/concourse")
if "/opt/trn_rl_repo" not in sys.path:
    sys.path.insert(0, "/opt/trn_rl_repo")

import concourse.bacc as bacc
import concourse.bass as bass
import concourse.mybir as mybir
from concourse import tile
from concourse.alu_op_type import AluOpType
from concourse.bass_utils import run_bass_kernel_spmd

F32 = mybir.dt.float32
AF = mybir.ActivationFunctionType

B, L, H = 2048, 128, 64
NCORES = 8
BC = B // NCORES  # 256 batch rows per core
D_REC, D_SP, D_DP, D_PR = 10, 128, 128, 8
G1 = 4   # sport/dport psum group (4*128 = 512 fp32 = one bank)
G2 = 8   # recon+proto psum group (8*2*18 = 288 <= 512)

# packed-constant column layout: name -> (rows, col0, ncol)
_PACK = {}
_ncol = 0


def _slot(name, rows, ncols):
    global _ncol
    _PACK[name] = (rows, _ncol, ncols)
    _ncol += ncols


_slot("ident", 128, 128)
_slot("wlhT", 128, H)
_slot("latT", 128, BC)
_slot("whh_rzT", H, 128)
_slot("wih_rzT", H, 128)
_slot("wspA", H + 1, D_SP)
_slot("wdpA", H + 1, D_DP)
_slot("whh_nT", H, H)
_slot("wih_nT", H, H)
_slot("wrpA", H + 1, D_REC + D_PR)
_slot("blh", H, 1)
_slot("bias_rz", 128, 1)
_slot("bias_gin", H, 1)
_slot("bias_hn", H, 1)
_slot("ones_bc", 1, BC)
NCOL = _ncol

_BUILD_CACHE = {}


def build_nc(S: int) -> bass.Bass:
    nc = bacc.Bacc("TRN2", target_bir_lowering=False, debug=False)

    wpack_d = nc.dram_tensor("wpack", [128, NCOL], F32, kind="ExternalInput")
    recon_d = nc.dram_tensor("recon", [BC, S, D_REC], F32, kind="ExternalOutput")
    sport_d = nc.dram_tensor("sport", [BC, S, D_SP], F32, kind="ExternalOutput")
    dport_d = nc.dram_tensor("dport", [BC, S, D_DP], F32, kind="ExternalOutput")
    proto_d = nc.dram_tensor("proto", [BC, S, D_PR], F32, kind="ExternalOutput")

    with tile.TileContext(nc) as tc:
        from contextlib import ExitStack

        with ExitStack() as ctx:
            const = ctx.enter_context(tc.tile_pool(name="const", bufs=1))
            sb = ctx.enter_context(tc.tile_pool(name="sb", bufs=2))
            hb = ctx.enter_context(tc.tile_pool(name="hb", bufs=3))
            stg = ctx.enter_context(tc.tile_pool(name="stg", bufs=2))
            p_rz = ctx.enter_context(tc.tile_pool(name="p_rz", bufs=1, space="PSUM"))
            p_n = ctx.enter_context(tc.tile_pool(name="p_n", bufs=1, space="PSUM"))
            p_sp0 = ctx.enter_context(tc.tile_pool(name="p_sp0", bufs=1, space="PSUM"))
            p_sp1 = ctx.enter_context(tc.tile_pool(name="p_sp1", bufs=1, space="PSUM"))
            p_dp0 = ctx.enter_context(tc.tile_pool(name="p_dp0", bufs=1, space="PSUM"))
            p_dp1 = ctx.enter_context(tc.tile_pool(name="p_dp1", bufs=1, space="PSUM"))
            p_rp = ctx.enter_context(tc.tile_pool(name="p_rp", bufs=1, space="PSUM"))

            wpack = const.tile([128, NCOL], F32, tag="wpack")
            nc.sync.dma_start(wpack[:], wpack_d[:])

            def cc(name):
                rows, c0, ncols = _PACK[name]
                return wpack[0:rows, c0 : c0 + ncols]

            ident, wlhT, latT = cc("ident"), cc("wlhT"), cc("latT")
            whh_rzT, wih_rzT = cc("whh_rzT"), cc("wih_rzT")
            wspA, wdpA = cc("wspA"), cc("wdpA")
            whh_nT, wih_nT, wrpA = cc("whh_nT"), cc("wih_nT"), cc("wrpA")
            blh, bias_rz = cc("blh"), cc("bias_rz")
            bias_gin, bias_hn = cc("bias_gin"), cc("bias_hn")
            ones_bc = cc("ones_bc")

            # ---- prologue: h0 = tanh(W_lh @ latent^T + b_lh), gi ----
            # two ping-pong h tiles, augmented with a constant 1.0 row used to
            # fold the head biases into the head matmuls (weights row H = bias)
            h_a = const.tile([H + 1, BC], F32, tag="h_a")
            h_b = const.tile([H + 1, BC], F32, tag="h_b")
            nc.vector.tensor_copy(h_a[H : H + 1, :], ones_bc)
            nc.vector.tensor_copy(h_b[H : H + 1, :], ones_bc)
            ph0 = p_n.tile([H, BC], F32, tag="n")
            nc.tensor.matmul(ph0[:], wlhT, latT, start=True, stop=True)
            nc.scalar.activation(h_a[0:H, :], ph0[:], AF.Tanh, bias=blh)
            cur, nxt = h_a, h_b
            h = cur[0:H, :]

            pgi = p_rz.tile([128, BC], F32, tag="rz")
            nc.tensor.matmul(pgi[:], wih_rzT, h, start=True, stop=True)
            gi_rz = const.tile([128, BC], F32, tag="gi_rz")
            nc.scalar.activation(gi_rz[:], pgi[:], AF.Identity, bias=bias_rz)

            pgn = p_n.tile([H, BC], F32, tag="n")
            nc.tensor.matmul(pgn[:], wih_nT, h, start=True, stop=True)
            gi_n = const.tile([H, BC], F32, tag="gi_n")
            nc.scalar.activation(gi_n[:], pgn[:], AF.Identity, bias=bias_gin)

            # ---- scan ----
            sp_ps = [None, None]
            dp_ps = [None, None]
            rp_ps = None
            g1_start = 0
            g2_start = 0

            for t in range(S):
                h = cur[0:H, :]
                prz = p_rz.tile([128, BC], F32, tag="rz")
                nc.tensor.matmul(prz[:], whh_rzT, h, start=True, stop=False)
                nc.tensor.matmul(prz[:], ident, gi_rz[:], start=False, stop=True)
                pn = p_n.tile([H, BC], F32, tag="n")
                nc.tensor.matmul(pn[:], whh_nT, h, start=True, stop=True)

                rz = sb.tile([128, BC], F32, tag="rz_sb")
                nc.scalar.activation(rz[:], prz[:], AF.Sigmoid)
                # z must live at base partition 0 for the DVE tensor-tensor ops
                z0 = sb.tile([H, BC], F32, tag="z0")
                nc.vector.tensor_copy(z0[:], rz[H:128, :])
                # t1 = (gh_n + b_hh_n) * r
                t1 = sb.tile([H, BC], F32, tag="t1")
                nc.vector.scalar_tensor_tensor(
                    t1[:], pn[:], bias_hn, rz[0:H, :],
                    op0=AluOpType.add, op1=AluOpType.mult,
                )
                t2 = sb.tile([H, BC], F32, tag="t2")
                nc.vector.tensor_add(t2[:], t1[:], gi_n[:])
                n_t = sb.tile([H, BC], F32, tag="n_sb")
                nc.scalar.activation(n_t[:], t2[:], AF.Tanh)
                d = sb.tile([H, BC], F32, tag="d")
                nc.vector.tensor_sub(d[:], h, n_t[:])
                m = sb.tile([H, BC], F32, tag="m")
                nc.vector.tensor_mul(m[:], z0[:], d[:])
                nc.vector.tensor_add(nxt[0:H, :], n_t[:], m[:])
                h_new = nxt

                # ---- heads ----
                r1 = t - g1_start
                r2 = t - g2_start
                if r1 == 0:
                    sp_ps[0] = p_sp0.tile([128, G1, D_SP], F32, tag="sp0", name=f"sp0_{t}")
                    sp_ps[1] = p_sp1.tile([128, G1, D_SP], F32, tag="sp1", name=f"sp1_{t}")
                    dp_ps[0] = p_dp0.tile([128, G1, D_DP], F32, tag="dp0", name=f"dp0_{t}")
                    dp_ps[1] = p_dp1.tile([128, G1, D_DP], F32, tag="dp1", name=f"dp1_{t}")
                if r2 == 0:
                    rp_ps = p_rp.tile([128, G2, 2, D_REC + D_PR], F32, tag="rp", name=f"rp_{t}")
                for c in range(2):
                    lhs = h_new[0 : H + 1, c * 128 : (c + 1) * 128]
                    nc.tensor.matmul(sp_ps[c][:, r1], lhs, wspA, start=True, stop=True)
                    nc.tensor.matmul(dp_ps[c][:, r1], lhs, wdpA, start=True, stop=True)
                    nc.tensor.matmul(rp_ps[:, r2, c], lhs, wrpA, start=True, stop=True)
                cur, nxt = nxt, cur

                if r1 == G1 - 1 or t == S - 1:
                    ng = r1 + 1
                    s0 = g1_start
                    stag_sp = stg.tile([128, 2, G1, D_SP], F32, tag="st_sp")
                    nc.scalar.copy(stag_sp[:, 0, 0:ng], sp_ps[0][:, 0:ng])
                    nc.vector.tensor_copy(stag_sp[:, 1, 0:ng], sp_ps[1][:, 0:ng])
                    dst = sport_d[:, s0 : s0 + ng, :].rearrange(
                        "(c p) t d -> p c t d", c=2
                    )
                    nc.sync.dma_start(dst, stag_sp[:, :, 0:ng])
                    stag_dp = stg.tile([128, 2, G1, D_DP], F32, tag="st_dp")
                    nc.vector.tensor_copy(stag_dp[:, 0, 0:ng], dp_ps[0][:, 0:ng])
                    nc.scalar.copy(stag_dp[:, 1, 0:ng], dp_ps[1][:, 0:ng])
                    dst = dport_d[:, s0 : s0 + ng, :].rearrange(
                        "(c p) t d -> p c t d", c=2
                    )
                    nc.sync.dma_start(dst, stag_dp[:, :, 0:ng])
                    g1_start = t + 1

                if r2 == G2 - 1 or t == S - 1:
                    ng = r2 + 1
                    s0 = g2_start
                    stag_rc = stg.tile([128, 2, G2, D_REC], F32, tag="st_rc")
                    src = rp_ps[:, 0:ng, :, 0:D_REC].rearrange("p t c d -> p c t d")
                    nc.scalar.copy(stag_rc[:, :, 0:ng], src)
                    dst = recon_d[:, s0 : s0 + ng, :].rearrange(
                        "(c p) t d -> p c t d", c=2
                    )
                    nc.sync.dma_start(dst, stag_rc[:, :, 0:ng])
                    stag_pr = stg.tile([128, 2, G2, D_PR], F32, tag="st_pr")
                    src = rp_ps[:, 0:ng, :, D_REC:].rearrange("p t c d -> p c t d")
                    nc.vector.tensor_copy(stag_pr[:, :, 0:ng], src)
                    dst = proto_d[:, s0 : s0 + ng, :].rearrange(
                        "(c p) t d -> p c t d", c=2
                    )
                    nc.sync.dma_start(dst, stag_pr[:, :, 0:ng])
                    g2_start = t + 1

    nc.compile()
    return nc


def _base_pack(W_lh, b_lh, W_ih, b_ih, W_hh, b_hh,
               W_out, b_out, W_sport, b_sport, W_dport, b_dport,
               W_proto, b_proto):
    f = np.float32
    pack = np.zeros((128, NCOL), dtype=f)

    def put(name, arr):
        rows, c0, ncols = _PACK[name]
        assert arr.shape == (rows, ncols), (name, arr.shape, (rows, ncols))
        pack[0:rows, c0 : c0 + ncols] = arr

    put("ident", np.eye(128, dtype=f))
    put("wlhT", W_lh.T.astype(f))
    put("whh_rzT", W_hh[0:128].T.astype(f))
    put("wih_rzT", W_ih[0:128].T.astype(f))
    put("wspA", np.vstack([W_sport.T, b_sport[None, :]]).astype(f))
    put("wdpA", np.vstack([W_dport.T, b_dport[None, :]]).astype(f))
    put("whh_nT", W_hh[128:192].T.astype(f))
    put("wih_nT", W_ih[128:192].T.astype(f))
    put("wrpA", np.vstack([
        np.concatenate([W_out.T, W_proto.T], axis=1),
        np.concatenate([b_out, b_proto])[None, :],
    ]).astype(f))
    put("blh", b_lh.astype(f).reshape(H, 1))
    put("bias_rz", (b_ih[0:128] + b_hh[0:128]).astype(f).reshape(128, 1))
    put("bias_gin", b_ih[128:192].astype(f).reshape(H, 1))
    put("bias_hn", b_hh[128:192].astype(f).reshape(H, 1))
    put("ones_bc", np.ones((1, BC), dtype=f))
    return pack


def kernel_with_results(**inputs):
    latent = np.asarray(inputs["latent"], dtype=np.float32)
    S = int(inputs["seq_len"])
    base = _base_pack(
        *(np.asarray(inputs[k]) for k in [
            "W_lh", "b_lh", "W_ih", "b_ih", "W_hh", "b_hh",
            "W_out", "b_out", "W_sport", "b_sport", "W_dport", "b_dport",
            "W_proto", "b_proto",
        ])
    )

    if S not in _BUILD_CACHE:
        _BUILD_CACHE[S] = build_nc(S)
    nc = _BUILD_CACHE[S]

    rows, c0, ncols = _PACK["latT"]
    in_maps = []
    for i in range(NCORES):
        p = base.copy()
        p[:, c0 : c0 + ncols] = latent[i * BC : (i + 1) * BC].T
        in_maps.append({"wpack": p})

    res = run_bass_kernel_spmd(nc, in_maps, core_ids=list(range(NCORES)))

    recon = np.concatenate([r["recon"] for r in res.results], axis=0)
    sport = np.concatenate([r["sport"] for r in res.results], axis=0)
    dport = np.concatenate([r["dport"] for r in res.results], axis=0)
    proto = np.concatenate([r["proto"] for r in res.results], axis=0)
    return (recon, sport, dport, proto), res


def kernel(**inputs):
    out, _ = kernel_with_results(**inputs)
    return out
